# revision 3
# baseline (speedup 1.0000x reference)
"""MetaGraphSAGE Trainium2 kernel (8 NeuronCores, Bass/Tile).

Per metagraph (3 independent graphs):
    h  = ELU(mean_agg(x) @ W1l + x @ W1r + b1)
    o  = mean_agg(h) @ W2l + h @ W2r + b2      (== mean_agg(h@W2l) + ...)
    out = log_softmax(o, axis=1)

Sharding: nodes padded to 8*6272=50176; core c owns dst nodes
[c*6272,(c+1)*6272). Edges are partitioned by dst owner on the host and
sorted by 128-wide dst block. On device, messages are gathered with
dma_gather (512B rows) and segment-summed via one-hot matmuls
accumulating in PSUM. Layer-2 gathers P2 = h@W2l (256B rows) from an
in-kernel AllGather output. int16 gather indices force a lo/hi split of
each block-group's edge list at src=32768.
"""

import sys

sys.path.insert(0, "/opt/trn_rl_repo")

import numpy as np

META, N, E, F, H, D = 3, 50000, 640000, 128, 128, 64
NCORES = 8
NSH = 6272            # padded nodes per core (6272*8 = 50176 >= N)
NPAD = NSH * NCORES
NBLK = NSH // 128     # 49 blocks of 128 dst nodes per core
GRP = 6               # dst blocks per PSUM group
WIN = 8               # max 128-edge chunks per dma_gather call (HW SWDGE cap ~1024 idx)
SPLIT = 32768         # int16 gather index limit
DMA_SCRATCH = 16384   # per-partition SWDGE descriptor carveout bytes
NQUEUE = 4            # SWDGE queues for gather round-robin (ucode max)
MAXSEG = 96           # max chunk cols in one (group, seg) metadata load
PHASE = 3             # debug: 1=L1 only, 2=+AllGather, 3=full


def _ceil(a, b):
    return (a + b - 1) // b


def _prep_host(meta_x, meta_edge_index):
    """Partition+sort edges per (core, graph); build all per-core arrays.

    Returns (layout, per_core) where layout holds compile-time constants
    (identical across cores) and per_core[c] holds numpy inputs.
    """
    meta_x = np.asarray(meta_x, dtype=np.float32)
    ei = np.asarray(meta_edge_index, dtype=np.int64)

    # --- per (core, graph, blk): edge lists split lo/hi by src ---
    # store per (c,g): list over stream of (blk, seg) chunk metadata
    counts_lo = np.zeros((NCORES, META, NBLK), dtype=np.int64)
    counts_hi = np.zeros((NCORES, META, NBLK), dtype=np.int64)
    edata = {}  # (c,g,blk,seg) -> (src_idx_array, d128_array)
    inv_all = np.zeros((NCORES, META, NSH), dtype=np.float32)
    for g in range(META):
        src = ei[g, 0]
        dst = ei[g, 1]
        core = dst // NSH
        for c in range(NCORES):
            sel = core == c
            s = src[sel]
            dl = dst[sel] - c * NSH
            cnt = np.bincount(dl, minlength=NSH).astype(np.float32)
            inv_all[c, g] = 1.0 / np.maximum(cnt, 1.0)
            blk = dl // 128
            d128 = dl % 128
            hi = (s >= SPLIT).astype(np.int64)
            order = np.lexsort((hi, blk))
            s, d128, blk, hi = s[order], d128[order], blk[order], hi[order]
            # boundaries per (blk, seg)
            for b in range(NBLK):
                mb = blk == b
                sb, db, hb = s[mb], d128[mb], hi[mb]
                lo_n = int((hb == 0).sum())
                counts_lo[c, g, b] = lo_n
                counts_hi[c, g, b] = len(sb) - lo_n
                edata[(c, g, b, 0)] = (sb[:lo_n], db[:lo_n])
                edata[(c, g, b, 1)] = (sb[lo_n:] - SPLIT, db[lo_n:])

    # padded chunk counts per (g, blk, seg): max over cores, in 128-chunks
    pl = np.maximum(_ceil_arr(counts_lo.max(axis=0), 128), 1)  # [META,NBLK]
    ph = np.maximum(_ceil_arr(counts_hi.max(axis=0), 128), 1)

    # --- stream layout per graph: groups of GRP blocks, [lo segs | hi segs]
    # chunk stream entries: (blk, seg) per 128-edge chunk, in order
    stream = [[] for _ in range(META)]   # per g: list of (blk, seg)
    groups = []  # list of (g, blk_start, nblk, lo_chunk0, lo_nchunk, hi_chunk0, hi_nchunk)
    for g in range(META):
        for b0 in range(0, NBLK, GRP):
            nb = min(GRP, NBLK - b0)
            lo0 = len(stream[g])
            for b in range(b0, b0 + nb):
                stream[g] += [(b, 0)] * int(pl[g, b])
            hi0 = len(stream[g])
            for b in range(b0, b0 + nb):
                stream[g] += [(b, 1)] * int(ph[g, b])
            groups.append((g, b0, nb, lo0, hi0 - lo0, hi0, len(stream[g]) - hi0))
    totc = [len(stream[g]) for g in range(META)]  # chunks per graph

    # --- per-core flat arrays in stream order ---
    per_core = []
    for c in range(NCORES):
        idx16 = [np.zeros((t * 128,), dtype=np.int16) for t in totc]
        dstf = [np.full((t * 128,), -1.0, dtype=np.float32) for t in totc]
        for g in range(META):
            pos = 0
            cur = None
            for (b, seg) in stream[g]:
                if cur != (b, seg):
                    cur = (b, seg)
                    sarr, darr = edata[(c, g, b, seg)]
                    off = 0
                n = min(128, len(sarr) - off) if len(sarr) > off else 0
                if n > 0:
                    idx16[g][pos : pos + n] = sarr[off : off + n].astype(np.int16)
                    dstf[g][pos : pos + n] = darr[off : off + n].astype(np.float32)
                off += 128
                pos += 128
        # wrap: idx i -> [i%16, i//16], replicated to 128 partitions
        idxw = np.concatenate(
            [np.tile(a.reshape(-1, 16).T, (8, 1)) for a in idx16], axis=1
        )  # [128, sum(totc)*8]
        dstw = np.concatenate(
            [a.reshape(-1, 128).T for a in dstf], axis=1
        )  # [128, sum(totc)]
        xts = np.zeros((META, 128, NSH), dtype=np.float32)
        lo = c * NSH
        hi_n = min(NSH, N - lo)
        for g in range(META):
            if hi_n > 0:
                xts[g, :, :hi_n] = meta_x[g, lo : lo + hi_n].T
        invb = np.broadcast_to(
            inv_all[c][:, None, :], (META, 128, NSH)
        ).copy()  # [META,128,NSH]
        invt = (
            inv_all[c].reshape(META, NBLK, 128).transpose(0, 2, 1).copy()
        )  # [META,128,NBLK]
        per_core.append(
            dict(idxw=idxw, dstw=dstw, xts=xts, invb=invb, invt=invt)
        )

    layout = dict(stream=stream, groups=groups, totc=totc, pl=pl, ph=ph)
    return layout, per_core


def _ceil_arr(a, b):
    return (a + b - 1) // b


def _build_program(layout):
    import concourse.bass as bass  # noqa: F401
    import concourse.mybir as mybir
    import concourse.tile as tile
    from concourse import bacc

    fp32 = mybir.dt.float32
    i16 = mybir.dt.int16
    nc = bacc.Bacc(None, dynamic_dma_scratch_size=DMA_SCRATCH, num_swdge_queues=NQUEUE)
    core_ids = list(range(NCORES))

    totc = layout["totc"]
    totc_all = sum(totc)

    # ---- dram I/O ----
    x_in = nc.declare_dram_parameter("x", [META, N, F], fp32, isOutput=False)
    idx_in = nc.declare_dram_parameter("idx", [128, totc_all * 8], i16, isOutput=False)
    dst_in = nc.declare_dram_parameter("dstw", [128, totc_all], fp32, isOutput=False)
    xts_in = nc.declare_dram_parameter("xts", [META, 128, NSH], fp32, isOutput=False)
    invb_in = nc.declare_dram_parameter("invb", [META, 128, NSH], fp32, isOutput=False)
    invt_in = nc.declare_dram_parameter("invt", [META, 128, NBLK], fp32, isOutput=False)
    w1l_in = nc.declare_dram_parameter("w1l", [META, F, H], fp32, isOutput=False)
    w1r_in = nc.declare_dram_parameter("w1r", [META, F, H], fp32, isOutput=False)
    b1_in = nc.declare_dram_parameter("b1c", [META, H, 1], fp32, isOutput=False)
    w2l_in = nc.declare_dram_parameter("w2l", [META, H, D], fp32, isOutput=False)
    w2r_in = nc.declare_dram_parameter("w2r", [META, H, D], fp32, isOutput=False)
    b2_in = nc.declare_dram_parameter("b2b", [META, 128, D], fp32, isOutput=False)
    iota_in = nc.declare_dram_parameter("iota", [128, 128], fp32, isOutput=False)
    out_ext = nc.declare_dram_parameter("out", [META, NSH, D], fp32, isOutput=True)

    p2shard = [nc.dram_tensor(f"p2shard{g}", [NSH, D], fp32) for g in range(META)]
    p2full = [
        nc.dram_tensor(f"p2full{g}", [NPAD, D], fp32, addr_space="Shared")
        for g in range(META)
    ]

    stream = layout["stream"]
    groups = layout["groups"]
    pl, ph = layout["pl"], layout["ph"]
    maxseg = max(max(gr[4], gr[6]) for gr in groups)

    # chunk -> global stream col (per graph, with graph offsets for dram slicing)
    goff = [sum(totc[:g]) for g in range(META)]

    with tile.TileContext(nc) as tc:
        with (
            tc.tile_pool(name="const", bufs=1) as cpool,
            tc.tile_pool(name="weights", bufs=1) as wpool,
            tc.tile_pool(name="hT", bufs=1) as hpool,
            tc.tile_pool(name="gath", bufs=4) as gpool,
            tc.tile_pool(name="oneh", bufs=4) as opool,
            tc.tile_pool(name="meta", bufs=4) as mpool,
            tc.tile_pool(name="dense", bufs=4) as dpool,
            tc.tile_pool(name="psA", bufs=1, space="PSUM") as psA,
            tc.tile_pool(name="psB", bufs=2, space="PSUM") as psB,
            tc.tile_pool(name="psC", bufs=1, space="PSUM") as psC,
            tc.tile_pool(name="psD", bufs=1, space="PSUM") as psD,
        ):
            iota_t = cpool.tile([128, 128], fp32, tag="iota")
            nc.sync.dma_start(out=iota_t[:], in_=iota_in[:])

            # resident per-graph constants
            w1l_t, w1r_t, w2l_t, w2r_t, b1_t, b2_t, invt_t = [], [], [], [], [], [], []
            for g in range(META):
                t = wpool.tile([F, H], fp32, tag=f"w1l{g}", name=f"w1l{g}")
                nc.sync.dma_start(out=t[:], in_=w1l_in[g])
                w1l_t.append(t)
                t = wpool.tile([F, H], fp32, tag=f"w1r{g}", name=f"w1r{g}")
                nc.sync.dma_start(out=t[:], in_=w1r_in[g])
                w1r_t.append(t)
                t = wpool.tile([H, D], fp32, tag=f"w2l{g}", name=f"w2l{g}")
                nc.sync.dma_start(out=t[:], in_=w2l_in[g])
                w2l_t.append(t)
                t = wpool.tile([H, D], fp32, tag=f"w2r{g}", name=f"w2r{g}")
                nc.sync.dma_start(out=t[:], in_=w2r_in[g])
                w2r_t.append(t)
                t = wpool.tile([H, 1], fp32, tag=f"b1{g}", name=f"b1{g}")
                nc.sync.dma_start(out=t[:], in_=b1_in[g])
                b1_t.append(t)
                t = wpool.tile([128, D], fp32, tag=f"b2{g}", name=f"b2{g}")
                nc.sync.dma_start(out=t[:], in_=b2_in[g])
                b2_t.append(t)
                t = wpool.tile([128, NBLK], fp32, tag=f"invt{g}", name=f"invt{g}")
                nc.sync.dma_start(out=t[:], in_=invt_in[g])
                invt_t.append(t)

            hT = [hpool.tile([H, NSH], fp32, tag=f"hTg{g}", name=f"hTg{g}") for g in range(META)]

            def edge_phase(g, layer):
                """Gather + one-hot matmul accumulate for all groups of graph g.

                layer 1: G rows from x[g] (128 wide), psum aggT [F x 128dst],
                         unswapped (lhsT=G, rhs=onehot).
                layer 2: G rows from p2full[g] (64 wide), psum agg [128dst x D],
                         swapped (lhsT=onehot, rhs=G).
                Calls blk_done(g, blk, psum_slice) when a block's psum is
                complete.
                """
                ew = F if layer == 1 else D
                qrr = [0]
                for (gg, b0, nb, lo0, lon, hi0, hin) in groups:
                    if gg != g:
                        continue
                    # two psum regions per block (lo/hi) so every matmul
                    # accumulation group is contiguous on PE (interleaved
                    # groups within a bank are broken on HW).
                    per_bank = 4 if layer == 1 else 8  # regions per bank
                    pool = psA if layer == 1 else psD
                    nbank = _ceil(nb * 2, per_bank)
                    ps = [
                        pool.tile([128, 512], fp32, tag=f"edge{layer}_{i}", name=f"ps{layer}_{i}")
                        for i in range(nbank)
                    ]

                    def psum_slice(b, seg):
                        j = (b - b0) * 2 + seg
                        return ps[j // per_bank][
                            :, (j % per_bank) * ew : (j % per_bank) * ew + ew
                        ]

                    started = set()
                    # chunk counts per (block, seg) for stop flags
                    left = {}
                    for b in range(b0, b0 + nb):
                        left[(b, 0)] = int(pl[g, b])
                        left[(b, 1)] = int(ph[g, b])

                    for (c0, ncols) in ((lo0, lon), (hi0, hin)):
                        seg = 0 if c0 == lo0 else 1
                        # gather source AP
                        if layer == 1:
                            sp = SPLIT if N > SPLIT else 0
                            src_ap = (
                                x_in[g, :, :] if seg == 0 else x_in[g, sp:, :]
                            )
                        else:
                            sp = SPLIT if NPAD > SPLIT else 0
                            src_ap = (
                                p2full[g][:, :] if seg == 0 else p2full[g][sp:, :]
                            )
                        # one idx/dst load per (group, seg)
                        gcol = goff[g] + c0
                        itg = mpool.tile([128, maxseg * 8], i16, tag="idx", name="itg")
                        nc.sync.dma_start(
                            out=itg[:, : ncols * 8],
                            in_=idx_in[:, gcol * 8 : (gcol + ncols) * 8],
                        )
                        dtg = mpool.tile([128, maxseg], fp32, tag="dst", name="dtg")
                        nc.sync.dma_start(
                            out=dtg[:, :ncols], in_=dst_in[:, gcol : gcol + ncols]
                        )
                        for w0 in range(c0, c0 + ncols, WIN):
                            wn = min(WIN, c0 + ncols - w0)
                            lc = w0 - c0
                            gt = gpool.tile([128, WIN, ew], fp32, tag="gt", name=f"gt{layer}")
                            nc.gpsimd.dma_gather(
                                gt[:, :wn, :],
                                src_ap,
                                itg[:, lc * 8 : (lc + wn) * 8],
                                wn * 128,
                                wn * 128,
                                ew,
                                queue_num=qrr[0] % NQUEUE,
                            )
                            qrr[0] += 1
                            oh = opool.tile([128, WIN, 128], fp32, tag="oh")
                            nc.vector.tensor_tensor(
                                out=oh[:, :wn, :],
                                in0=dtg[:, lc : lc + wn]
                                .rearrange("p (w o) -> p w o", o=1)
                                .to_broadcast([128, wn, 128]),
                                in1=iota_t[:]
                                .rearrange("p (o d) -> p o d", o=1)
                                .to_broadcast([128, wn, 128]),
                                op=mybir.AluOpType.is_equal,
                            )
                            for j in range(wn):
                                b, sseg = stream[g][w0 + j]
                                first = (b, sseg) not in started
                                if first:
                                    started.add((b, sseg))
                                left[(b, sseg)] -= 1
                                if layer == 1:
                                    nc.tensor.matmul(
                                        out=psum_slice(b, sseg),
                                        lhsT=gt[:, j, :],
                                        rhs=oh[:, j, :],
                                        start=first,
                                        stop=left[(b, sseg)] == 0,
                                        skip_group_check=True,
                                    )
                                else:
                                    nc.tensor.matmul(
                                        out=psum_slice(b, sseg),
                                        lhsT=oh[:, j, :],
                                        rhs=gt[:, j, :],
                                        start=first,
                                        stop=left[(b, sseg)] == 0,
                                        skip_group_check=True,
                                    )
                    for b in range(b0, b0 + nb):
                        yield b, psum_slice(b, 0), psum_slice(b, 1)

            # ================= per graph =================
            for g in range(META):
                # ---- layer 1 ----
                grp_cache = {}
                for b, aggL, aggH in edge_phase(g, 1):
                    b0g = (b // GRP) * GRP
                    if b0g not in grp_cache:
                        nbg = min(GRP, NBLK - b0g) * 128
                        ibg = mpool.tile([128, GRP * 128], fp32, tag="invbg", name="ibg")
                        nc.sync.dma_start(
                            out=ibg[:, :nbg],
                            in_=invb_in[g, :, b0g * 128 : b0g * 128 + nbg],
                        )
                        xtg = mpool.tile([128, GRP * 128], fp32, tag="xtsg", name="xtg")
                        nc.sync.dma_start(
                            out=xtg[:, :nbg],
                            in_=xts_in[g, :, b0g * 128 : b0g * 128 + nbg],
                        )
                        grp_cache = {b0g: (ibg, xtg)}
                    ibg, xtg = grp_cache[b0g]
                    boff = (b - b0g) * 128
                    # mean1T = (aggL + aggH) * invb  [F x 128dst]
                    sc = dpool.tile([F, 128], fp32, tag="sc")
                    nc.vector.tensor_copy(out=sc[:], in_=aggL)
                    s0 = dpool.tile([F, 128], fp32, tag="s0")
                    nc.vector.tensor_tensor(
                        out=s0[:], in0=aggH, in1=sc[:], op=mybir.AluOpType.add
                    )
                    m1 = dpool.tile([F, 128], fp32, tag="m1")
                    nc.vector.tensor_tensor(
                        out=m1[:],
                        in0=s0[:],
                        in1=ibg[:, boff : boff + 128],
                        op=mybir.AluOpType.mult,
                    )
                    xt = xtg[:, boff : boff + 128]
                    o1 = psB.tile([H, 128], fp32, tag="work", name="o1")
                    nc.tensor.matmul(
                        out=o1[:], lhsT=w1l_t[g][:], rhs=m1[:], start=True, stop=False
                    )
                    nc.tensor.matmul(
                        out=o1[:], lhsT=w1r_t[g][:], rhs=xt, start=False, stop=True
                    )
                    # ELU: h = max(z,0) + exp(min(z,0)) - 1, z = o1 + b1
                    tm = dpool.tile([H, 128], fp32, tag="tm")
                    nc.vector.tensor_scalar(
                        out=tm[:],
                        in0=o1[:],
                        scalar1=b1_t[g][:, :1],
                        scalar2=0.0,
                        op0=mybir.AluOpType.add,
                        op1=mybir.AluOpType.min,
                    )
                    tp = dpool.tile([H, 128], fp32, tag="tp")
                    nc.vector.tensor_scalar(
                        out=tp[:],
                        in0=o1[:],
                        scalar1=b1_t[g][:, :1],
                        scalar2=0.0,
                        op0=mybir.AluOpType.add,
                        op1=mybir.AluOpType.max,
                    )
                    te = dpool.tile([H, 128], fp32, tag="te")
                    nc.scalar.activation(
                        out=te[:], in_=tm[:], func=mybir.ActivationFunctionType.Exp
                    )
                    ts_ = dpool.tile([H, 128], fp32, tag="ts")
                    nc.vector.tensor_tensor(
                        out=ts_[:], in0=te[:], in1=tp[:], op=mybir.AluOpType.add
                    )
                    hs = hT[g][:, b * 128 : b * 128 + 128]
                    nc.vector.tensor_scalar(
                        out=hs,
                        in0=ts_[:],
                        scalar1=-1.0,
                        scalar2=None,
                        op0=mybir.AluOpType.add,
                    )
                    # P2 block = h_b @ W2l  -> [128dst x D]
                    p2p = psB.tile([128, D], fp32, tag="work", name="p2p")
                    nc.tensor.matmul(
                        out=p2p[:], lhsT=hs, rhs=w2l_t[g][:], start=True, stop=True
                    )
                    p2s = dpool.tile([128, D], fp32, tag="p2s")
                    nc.vector.tensor_copy(out=p2s[:], in_=p2p[:])
                    nc.sync.dma_start(
                        out=p2shard[g][b * 128 : b * 128 + 128, :], in_=p2s[:]
                    )
                    if PHASE in (1, 2):
                        nc.sync.dma_start(
                            out=out_ext[g, b * 128 : b * 128 + 128, :], in_=p2s[:]
                        )

                if PHASE == 1:
                    continue
                # ---- allgather P2 ----
                nc.gpsimd.collective_compute(
                    "AllGather",
                    mybir.AluOpType.bypass,
                    ins=[p2shard[g][:]],
                    outs=[p2full[g][:]],
                    replica_groups=[core_ids],
                )

            # ================= layer 2, all graphs =================
            for g in range(META):
                if PHASE in (1, 2):
                    continue
                # ---- layer 2 ----
                for b, agg2L, agg2H in edge_phase(g, 2):
                    # mean2 = (agg2L + agg2H) * invT  [128dst x D]
                    s2c = dpool.tile([128, D], fp32, tag="s2c")
                    nc.vector.tensor_copy(out=s2c[:], in_=agg2L)
                    s2 = dpool.tile([128, D], fp32, tag="s2")
                    nc.vector.tensor_tensor(
                        out=s2[:], in0=agg2H, in1=s2c[:], op=mybir.AluOpType.add
                    )
                    t1 = dpool.tile([128, D], fp32, tag="t1")
                    nc.vector.tensor_scalar(
                        out=t1[:],
                        in0=s2[:],
                        scalar1=invt_t[g][:, b : b + 1],
                        scalar2=None,
                        op0=mybir.AluOpType.mult,
                    )
                    o2 = psC.tile([128, D], fp32, tag="o2", name="o2")
                    nc.tensor.matmul(
                        out=o2[:],
                        lhsT=hT[g][:, b * 128 : b * 128 + 128],
                        rhs=w2r_t[g][:],
                        start=True,
                        stop=True,
                    )
                    t2 = dpool.tile([128, D], fp32, tag="t2")
                    nc.vector.tensor_tensor(
                        out=t2[:], in0=t1[:], in1=o2[:], op=mybir.AluOpType.add
                    )
                    t3 = dpool.tile([128, D], fp32, tag="t3")
                    nc.vector.tensor_tensor(
                        out=t3[:], in0=t2[:], in1=b2_t[g][:], op=mybir.AluOpType.add
                    )
                    # log_softmax along free dim (D)
                    rmax = dpool.tile([128, 1], fp32, tag="rmax")
                    nc.vector.reduce_max(out=rmax[:], in_=t3[:], axis=mybir.AxisListType.X)
                    x1 = dpool.tile([128, D], fp32, tag="x1")
                    nc.vector.tensor_scalar(
                        out=x1[:],
                        in0=t3[:],
                        scalar1=rmax[:, :1],
                        scalar2=None,
                        op0=mybir.AluOpType.subtract,
                    )
                    ex = dpool.tile([128, D], fp32, tag="ex")
                    nc.scalar.activation(
                        out=ex[:], in_=x1[:], func=mybir.ActivationFunctionType.Exp
                    )
                    sm = dpool.tile([128, 1], fp32, tag="sm")
                    nc.vector.reduce_sum(out=sm[:], in_=ex[:], axis=mybir.AxisListType.X)
                    ls = dpool.tile([128, 1], fp32, tag="ls")
                    nc.scalar.activation(
                        out=ls[:], in_=sm[:], func=mybir.ActivationFunctionType.Ln
                    )
                    ob = dpool.tile([128, D], fp32, tag="ob")
                    nc.vector.tensor_scalar(
                        out=ob[:],
                        in0=x1[:],
                        scalar1=ls[:, :1],
                        scalar2=None,
                        op0=mybir.AluOpType.subtract,
                    )
                    nc.sync.dma_start(
                        out=out_ext[g, b * 128 : b * 128 + 128, :], in_=ob[:]
                    )

    nc.finalize()
    return nc


def kernel(**inputs):
    out, _ = run_kernel(inputs)
    return out


def run_for_test(inputs, bench):
    """Devloop entry: one compile, correctness outs + chained timing."""
    nc, in_maps = _build_all(inputs)
    results, per_exec_ns = bench.run_and_bench(nc, in_maps, NCORES)
    return _unshard(results), per_exec_ns


def _unshard(results):
    out = np.zeros((META, N, D), dtype=np.float32)
    for c in range(NCORES):
        lo = c * NSH
        n = min(NSH, N - lo)
        out[:, lo : lo + n, :] = results[c]["out"][:, :n, :]
    return out


def run_kernel(inputs, trace=False):
    from concourse.bass_utils import run_bass_kernel_spmd

    nc, in_maps = _build_all(inputs)
    res = run_bass_kernel_spmd(nc, in_maps, list(range(NCORES)), trace=trace)
    return _unshard(res.results), res


def _build_all(inputs):
    meta_x = np.asarray(inputs["meta_x"], dtype=np.float32)
    layout, per_core = _prep_host(meta_x, inputs["meta_edge_index"])
    nc = _build_program(layout)

    iota = np.tile(np.arange(128, dtype=np.float32), (128, 1))
    w1l = np.asarray(inputs["W1l"], dtype=np.float32)
    w1r = np.asarray(inputs["W1r"], dtype=np.float32)
    w2l = np.asarray(inputs["W2l"], dtype=np.float32)
    w2r = np.asarray(inputs["W2r"], dtype=np.float32)
    b1 = np.asarray(inputs["b1"], dtype=np.float32)
    b2 = np.asarray(inputs["b2"], dtype=np.float32)
    b1c = b1[:, :, None].copy()
    b2b = np.broadcast_to(b2[:, None, :], (META, 128, D)).copy()

    in_maps = []
    for c in range(NCORES):
        pc = per_core[c]
        in_maps.append(
            {
                "x": meta_x,
                "idx": pc["idxw"],
                "dstw": pc["dstw"],
                "xts": pc["xts"],
                "invb": pc["invb"],
                "invt": pc["invt"],
                "w1l": w1l,
                "w1r": w1r,
                "b1c": b1c,
                "w2l": w2l,
                "w2r": w2r,
                "b2b": b2b,
                "iota": iota,
            }
        )

    return nc, in_maps

